# revision 23
# baseline (speedup 1.0000x reference)
"""ActionDecoder (img-conditioned LSTM + head) Trainium2 kernel.

Full inputs -> full outputs. Data-parallel over batch across 8 NeuronCores
(8 batch rows per core, weights replicated). The T=512 recurrence runs
locally per core in a transposed layout:

  state  hT/cT: [128 partitions (h-unit within k-chunk), 4 k-chunks * 8 batch]
  gates: 4 PSUM tiles [128, 32] (gate order g,i,f,o), col = ktile*8 + b

Per step: 64 self-loading bf16 matmuls (stationary = W_hh^T 128x128 tiles,
moving = hT chunk [128,8]) + DVE/ACT elementwise. The input-side projection
(img part + token embedding part + biases) is precomputed for all timesteps
into SBUF (gxeT) before the loop.
"""

import sys
import numpy as np

sys.path.insert(0, "/opt/trn_rl_repo")

import concourse.bass as bass
import concourse.bacc as bacc
import concourse.tile as tile
from concourse import mybir
from concourse.bass_utils import run_bass_kernel_spmd

import ml_dtypes

BF16 = ml_dtypes.bfloat16

B, T, V, E, IMG, H, A = 64, 512, 512, 128, 1024, 512, 512
NCORE = 8
BL = B // NCORE          # batch per core = 8
NKC = H // 128           # 4 k-chunks of hidden dim
NS = (4 * H) // 128      # 16 gate m-tiles
NIC = IMG // 128         # 8 img k-chunks
NAT = A // 128           # 4 head out tiles

# gate column order in PSUM: g, i, f, o  (so g finishes first, o last)
# torch W row-blocks: i=0, f=1, g=2, o=3
GATE_BLOCK = [2, 0, 1, 3]

F32 = mybir.dt.float32
BF = mybir.dt.bfloat16
I32 = mybir.dt.int32


def _rows_perm():
    rows = []
    for s in range(NS):
        base = GATE_BLOCK[s // 4] * H + (s % 4) * 128
        rows.append(np.arange(base, base + 128))
    return np.concatenate(rows)  # [2048]


def build_program(t_steps=T, unroll=8, rep=1):
    # Bacc (not Bass): its compile() splits multi-sem waits into
    # EventSemaphore instructions — walrus caps non-event instructions
    # at ONE sync wait and errors otherwise.
    nc = bacc.Bacc()

    # ---- DRAM parameters (per-core inputs; weights identical across cores)
    whhT_d = nc.declare_dram_parameter("whhT", [128, NKC * NS * 128], BF, isOutput=False)
    wiT_d = nc.declare_dram_parameter("wiT", [128, NIC * NS * 128], BF, isOutput=False)
    weT_d = nc.declare_dram_parameter("weT", [128, NS * 128], BF, isOutput=False)
    wactT_d = nc.declare_dram_parameter("wactT", [128, NKC * NAT * 128], BF, isOutput=False)
    bias2x_d = nc.declare_dram_parameter("bias2x", [128, NS * BL], F32, isOutput=False)
    bactx_d = nc.declare_dram_parameter("bactx", [128, NAT * BL], F32, isOutput=False)
    ident_d = nc.declare_dram_parameter("ident", [128, 128], F32, isOutput=False)
    emb_d = nc.declare_dram_parameter("emb", [V, E], F32, isOutput=False)
    x1T_d = nc.declare_dram_parameter("x1T", [128, NIC * BL], BF, isOutput=False)
    x2g_d = nc.declare_dram_parameter("x2g", [128, (t_steps * BL) // 128], I32, isOutput=False)
    lens_d = nc.declare_dram_parameter("lens", [1, BL], I32, isOutput=False)
    out_d = nc.declare_dram_parameter("out", [128, NAT * BL], F32, isOutput=True)

    # internal DRAM: h history [t, p, c] (c = ktile*8 + b), bf16
    hs_d = nc.dram_tensor("hs", [t_steps, 128, NKC * BL], BF)

    NTB = (t_steps * BL) // 128  # number of 128-row gather tiles (=32 @T=512)
    SIG = mybir.ActivationFunctionType.Sigmoid
    TANH = mybir.ActivationFunctionType.Tanh

    with tile.TileContext(nc) as tc:
        with tc.tile_pool(name="const", bufs=1) as cpool:
            whhT = cpool.tile([128, NKC * NS * 128], BF)
            weT = cpool.tile([128, NS * 128], BF)
            wactT = cpool.tile([128, NKC * NAT * 128], BF)
            bias2x = cpool.tile([128, NS * BL], F32)
            bactx = cpool.tile([128, NAT * BL], F32)
            ident = cpool.tile([128, 128], F32)
            x1T = cpool.tile([128, NIC * BL], BF)
            idx = cpool.tile([128, NTB], I32)
            lens_sb = cpool.tile([1, BL], I32)
            tokT = cpool.tile([128, t_steps * BL], BF)
            gxcT = cpool.tile([128, NS * BL], F32)
            gxeT = cpool.tile([128, t_steps * NS * BL], BF)
            # persistent state (hT = ring of `unroll` slots, each NKC*BL cols)
            hT = cpool.tile([128, unroll * NKC * BL], BF)
            cT = cpool.tile([128, NKC * BL], F32)
            hnT = cpool.tile([128, NKC * BL], BF)
            out_sb = cpool.tile([128, NAT * BL], F32)

            nc.sync.dma_start(out=whhT[:], in_=whhT_d[:])
            nc.sync.dma_start(out=weT[:], in_=weT_d[:])
            nc.sync.dma_start(out=wactT[:], in_=wactT_d[:])
            nc.sync.dma_start(out=bias2x[:], in_=bias2x_d[:])
            nc.sync.dma_start(out=bactx[:], in_=bactx_d[:])
            nc.sync.dma_start(out=ident[:], in_=ident_d[:])
            nc.sync.dma_start(out=x1T[:], in_=x1T_d[:])
            nc.sync.dma_start(out=idx[:], in_=x2g_d[:])
            nc.sync.dma_start(out=lens_sb[:], in_=lens_d[:])

            nc.vector.memset(hT[:], 0.0)
            nc.vector.memset(cT[:], 0.0)

            # ---------- prologue ----------
            with (
                tc.tile_pool(name="pro_sb", bufs=2) as ppool,
                tc.tile_pool(name="pro_tok", bufs=4) as tpool,
                tc.tile_pool(name="pro_ps", bufs=2, space="PSUM") as pps,
                tc.tile_pool(name="pro_ps2", bufs=2, space="PSUM") as pps2,
            ):
                # gxc: img-side projection + biases -> [128, NS*BL] f32
                # wiT_d layout: [p, s, ic, mm]; one DMA + one psum group per s
                gxc_ps = pps.tile([128, NS * BL], F32)
                for s in range(NS):
                    wi_s = ppool.tile([128, NIC * 128], BF, tag="wi_s", name="wi_s")
                    # gpsimd (SWDGE): DIRECT2D HW-DGE DMAs only support 2 sync
                    # waits in this walrus build; slot-recycling loads carry 3
                    nc.gpsimd.dma_start(
                        out=wi_s[:], in_=wiT_d[:, s * NIC * 128:(s + 1) * NIC * 128]
                    )
                    for ic in range(NIC):
                        nc.tensor.matmul(
                            gxc_ps[:, s * BL:(s + 1) * BL],
                            lhsT=wi_s[:, ic * 128:(ic + 1) * 128],
                            rhs=x1T[:, ic * BL:(ic + 1) * BL],
                            start=(ic == 0),
                            stop=(ic == NIC - 1),
                        )
                nc.vector.tensor_add(gxcT[:], gxc_ps[:], bias2x[:])

                # token gather + transpose: tokT[e, t*BL+b] = emb[x2[b,t], e]
                for r in range(NTB):
                    tok_sb = tpool.tile([128, E], F32, tag="tok_sb")
                    nc.gpsimd.indirect_dma_start(
                        out=tok_sb[:],
                        out_offset=None,
                        in_=emb_d[:],
                        in_offset=bass.IndirectOffsetOnAxis(ap=idx[:, r:r + 1], axis=0),
                    )
                    tp_ps = pps2.tile([128, 128], F32, tag="tp_ps")
                    nc.tensor.transpose(out=tp_ps[:], in_=tok_sb[:], identity=ident[:])
                    nc.vector.tensor_copy(tokT[:, r * 128:(r + 1) * 128], tp_ps[:])

                # gxeT[:, t*128 + s*8 + b] = (W_e^T tok)[s-tile] + gxc
                GBLK = min(512, t_steps * BL)  # moving cols per matmul
                TBLK = GBLK // BL              # timesteps per matmul
                nblk = (t_steps * BL) // GBLK
                for s in range(NS):
                    for tb in range(nblk):
                        ge_ps = pps.tile([128, GBLK], F32, tag="ge_ps")
                        nc.tensor.matmul(
                            ge_ps[:],
                            lhsT=weT[:, s * 128:(s + 1) * 128],
                            rhs=tokT[:, tb * GBLK:(tb + 1) * GBLK],
                            start=True,
                            stop=True,
                        )
                        dst = bass.AP(
                            gxeT.tensor,
                            gxeT[:].offset + tb * TBLK * (NS * BL) + s * BL,
                            [gxeT[:].ap[0], [NS * BL, TBLK], [1, BL]],
                        )
                        src_b = bass.AP(
                            gxcT.tensor,
                            gxcT[:].offset + s * BL,
                            [gxcT[:].ap[0], [0, TBLK], [1, BL]],
                        )
                        nc.vector.tensor_add(dst, ge_ps[:], src_b)

            # ---------- recurrence ----------
            # hT is a ring of `unroll` slots so all in-body slicing is static;
            # only 2 dynamic access patterns per body (gxe stage + hs DMA).
            GW = 4 * BL          # one gate group = 32 cols
            SW = NS * BL         # per-step gxe slice = 128 cols
            nb = t_steps // unroll
            hs_pt = hs_d[:].rearrange("t p c -> p t c")

            with (
                tc.tile_pool(name="gps", bufs=1, space="PSUM") as gpool,
                tc.tile_pool(name="loop_sb", bufs=2) as lpool,
            ):
                g_ps = [
                    gpool.tile([128, 4 * BL], F32, tag=f"g{g}", name=f"g_ps{g}")
                    for g in range(4)
                ]

                def step(u, gstage):
                    up = (u - 1) % unroll
                    # 64 matmuls: gate g uses s-slots 4g..4g+3
                    for s in range(NS):
                        g = s // 4
                        for kc in range(NKC):
                            nc.tensor.matmul(
                                g_ps[g][:, (s % 4) * BL:(s % 4 + 1) * BL],
                                lhsT=whhT[:, (kc * NS + s) * 128:(kc * NS + s + 1) * 128],
                                rhs=hT[:, up * (NKC * BL) + kc * BL:
                                        up * (NKC * BL) + (kc + 1) * BL],
                                start=(kc == 0),
                                stop=(kc == NKC - 1),
                            )
                    acts = lpool.tile([128, NS * BL], F32, tag="acts", name="acts")
                    for g in range(4):
                        gsb = lpool.tile([128, GW], F32, tag=f"gsb{g}", name="gsb")
                        nc.vector.tensor_add(
                            gsb[:], g_ps[g][:],
                            gstage[:, u * SW + g * GW:u * SW + (g + 1) * GW],
                        )
                        nc.scalar.activation(
                            acts[:, g * GW:(g + 1) * GW], gsb[:],
                            TANH if g == 0 else SIG,
                        )
                    ig = lpool.tile([128, GW], F32, tag="ig", name="ig")
                    fc = lpool.tile([128, GW], F32, tag="fc", name="fc")
                    thc = lpool.tile([128, GW], F32, tag="thc", name="thc")
                    nc.vector.tensor_mul(ig[:], acts[:, GW:2 * GW], acts[:, 0:GW])
                    nc.vector.tensor_mul(fc[:], acts[:, 2 * GW:3 * GW], cT[:])
                    nc.vector.tensor_add(cT[:], ig[:], fc[:])
                    nc.scalar.activation(thc[:], cT[:], TANH)
                    nc.vector.tensor_mul(
                        hT[:, u * (NKC * BL):(u + 1) * (NKC * BL)],
                        acts[:, 3 * GW:4 * GW], thc[:],
                    )

                # NOTE: hint_engines=(PE,) faults the device on this runtime
                # (NRT_EXEC_UNIT_UNRECOVERABLE) — leave branch hints off.
                # rep>1 repeats the whole recurrence (timing builds only).
                with tc.For_i(0, nb * rep, 1) as tb:
                    tbm = tb % nb if rep > 1 else tb
                    gstage = lpool.tile([128, unroll * SW], BF, tag="gstage",
                                        name="gstage")
                    nc.vector.tensor_copy(
                        gstage[:], gxeT[:, bass.ds(tbm * (unroll * SW), unroll * SW)]
                    )
                    for u in range(unroll):
                        step(u, gstage)
                    nc.gpsimd.dma_start(
                        out=hs_pt[:, bass.ds(tbm * unroll, unroll), :],
                        in_=hT[:].rearrange("p (u c) -> p u c", c=NKC * BL),
                    )

            # ---------- epilogue: hn gather + head ----------
            # lens_sb holds len-1 (host-precomputed). Spread the 8 dynamic
            # gather DMAs across 4 engines to stay within per-engine registers.
            eng_map = [
                (mybir.EngineType.SP, nc.sync, (0, 1, 2)),
                (mybir.EngineType.Activation, nc.scalar, (3, 4, 5)),
                (mybir.EngineType.Pool, nc.gpsimd, (6, 7)),
            ]
            hsv2 = hs_d[:].rearrange("t p (kc b) -> t p kc b", b=BL)
            for etype, eng, bs in eng_map:
                _, len_vals = nc.values_load_multi_w_load_instructions(
                    lens_sb[0:1, bs[0]:bs[-1] + 1],
                    engines=(etype,),
                    min_val=0, max_val=t_steps - 1,
                    skip_runtime_bounds_check=True,
                )
                for j, b in enumerate(bs):
                    eng.dma_start(
                        out=hnT[:, b * NKC:(b + 1) * NKC],
                        in_=hsv2[bass.ds(len_vals[j], 1), :, :, b],
                    )
            hn_r = hnT[:].rearrange("p (b kc) -> p kc b", kc=NKC)
            with tc.tile_pool(name="head_ps", bufs=1, space="PSUM") as hps:
                nt_ps = hps.tile([128, NAT * BL], F32)
                for at in range(NAT):
                    for kc in range(NKC):
                        nc.tensor.matmul(
                            nt_ps[:, at * BL:(at + 1) * BL],
                            lhsT=wactT[:, (kc * NAT + at) * 128:(kc * NAT + at + 1) * 128],
                            rhs=hn_r[:, kc, :],
                            start=(kc == 0),
                            stop=(kc == NKC - 1),
                        )
                nc.vector.tensor_add(out_sb[:], nt_ps[:], bactx[:])
            nc.sync.dma_start(out=out_d[:], in_=out_sb[:])

    nc.compile()
    return nc


def pack_weights(emb, W_ih, W_hh, b_ih, b_hh, W_act, b_act):
    perm = _rows_perm()
    W_ih = np.asarray(W_ih, np.float32)
    W_hh = np.asarray(W_hh, np.float32)
    # whhT[p, kc, s, mm] = W_hh[perm[s*128+mm], kc*128+p]
    whh_p = W_hh[perm].reshape(NS, 128, NKC, 128)          # [s, mm, kc, p]
    whhT = np.ascontiguousarray(whh_p.transpose(3, 2, 0, 1)).reshape(128, -1).astype(BF16)
    we_p = W_ih[perm, IMG:].reshape(NS, 128, E)            # [s, mm, e]
    weT = np.ascontiguousarray(we_p.transpose(2, 0, 1)).reshape(128, -1).astype(BF16)
    wi_p = W_ih[perm, :IMG].reshape(NS, 128, NIC, 128)     # [s, mm, ic, p]
    wiT = np.ascontiguousarray(wi_p.transpose(3, 0, 2, 1)).reshape(128, -1).astype(BF16)
    bias2 = (np.asarray(b_ih, np.float32) + np.asarray(b_hh, np.float32))[perm]
    bias2x = np.ascontiguousarray(
        np.broadcast_to(bias2.reshape(NS, 128).T[:, :, None], (128, NS, BL))
    ).reshape(128, -1).astype(np.float32)
    wa = np.asarray(W_act, np.float32).reshape(NAT, 128, NKC, 128)  # [at, aa, kc, p]
    wactT = np.ascontiguousarray(wa.transpose(3, 2, 0, 1)).reshape(128, -1).astype(BF16)
    bactx = np.ascontiguousarray(
        np.broadcast_to(
            np.asarray(b_act, np.float32).reshape(NAT, 128).T[:, :, None],
            (128, NAT, BL),
        )
    ).reshape(128, -1).astype(np.float32)
    return dict(
        whhT=whhT, wiT=wiT, weT=weT, wactT=wactT, bias2x=bias2x, bactx=bactx,
        ident=np.eye(128, dtype=np.float32),
        emb=np.asarray(emb, np.float32),
    )


def pack_core_inputs(x1_l, x2_l, lens_l, t_steps=T):
    # x1T[p, ic*BL + b] = x1_l[b, ic*128+p]
    x1T = np.ascontiguousarray(
        np.asarray(x1_l, np.float32).reshape(BL, NIC, 128).transpose(2, 1, 0)
    ).reshape(128, NIC * BL).astype(BF16)
    x2f = np.asarray(x2_l, np.int64).T[:t_steps].reshape(-1)  # flat = t*BL + b
    ntb = (t_steps * BL) // 128
    x2g = np.ascontiguousarray(x2f.reshape(ntb, 128).T).astype(np.int32)
    lens = (np.asarray(lens_l, np.int64).reshape(1, BL) - 1).astype(np.int32)
    return dict(x1T=x1T, x2g=x2g, lens=lens)


def unpack_out(out_np):
    # out[aa, at*BL + b] -> nt[b, at*128+aa]
    return np.ascontiguousarray(
        out_np.reshape(128, NAT, BL).transpose(2, 1, 0)
    ).reshape(BL, A)


_CACHE = {}


def kernel(x1, x2, x2_lens, emb, W_ih, W_hh, b_ih, b_hh, W_act, b_act):
    if "nc" not in _CACHE:
        _CACHE["nc"] = build_program()
    nc = _CACHE["nc"]
    shared = pack_weights(emb, W_ih, W_hh, b_ih, b_hh, W_act, b_act)
    in_maps = []
    for c in range(NCORE):
        m = dict(shared)
        m.update(pack_core_inputs(
            np.asarray(x1)[c * BL:(c + 1) * BL],
            np.asarray(x2)[c * BL:(c + 1) * BL],
            np.asarray(x2_lens)[c * BL:(c + 1) * BL],
        ))
        in_maps.append(m)
    res = run_bass_kernel_spmd(nc, in_maps, list(range(NCORE)))
    out = np.concatenate(
        [unpack_out(res.results[c]["out"]) for c in range(NCORE)], axis=0
    )
    return out.astype(np.float32)
